# revision 11
# baseline (speedup 1.0000x reference)
"""Block-diagonal ZF equalizer (nn_BDEqualizer) as a Trainium2 Bass kernel.

Math: for every resource element (b, s, f) and UE u, solve the 8x8 complex
system H_u x_u = y_u where H_u[i, j] = h[b, 0, 8u+i, u, j, s, f] and
y_u[i] = y[b, 0, 8u+i, s, f].  Output x as [B, 1, 32, S, F, 2] (re/im last).

Strategy (data-parallel over the fft axis, per the sharding hint):
  - 8 cores, each owns a contiguous 128-subcarrier slice of F=1024.
  - Host pre-extracts the block-diagonal channel blocks (pure indexing) and
    ships per-core shards hd[B, U, 8, 8, S, 128] / yd[B, U, 8, S, 128].
  - On-chip layout: subcarriers on the 128 SBUF partitions, the other RE
    axes (u, b-pair, s) = 112 along the free dim.  Each of the 9 augmented
    matrix columns (8 of H + rhs) is a "plane" of 8 rows; every Gaussian
    elimination step is a full-width elementwise op, with per-RE pivot
    reciprocals.  Unpivoted LU + Jordan back-substitution, complex
    arithmetic as separate re/im tiles.
  - The 112 RE columns are split across TWO elementwise engines that run
    the whole solve independently on disjoint column blocks held in
    separate supertiles: DVE (~1.04 ns/elem fp32) takes ND columns and
    Pool/GPSIMD (~1.98 ns/elem via TensorTensor) takes the rest.  Pool has
    no reciprocal or scalar_tensor_tensor, so pivot reciprocals use a
    ones/x TensorTensor divide and factors are computed sign-positive
    (G = +H[i,k]*inv(p)) so only plain add/sub/mult TT ops are needed.
  - TensorE transposes move between the DMA-friendly [(u,b,s), f] staging
    layout and the compute layout [f, (u,b,s)]; ScalarE drains PSUM into
    the per-engine supertiles and computes the pivot |p|^2 squares.
  - Two chunks (b in {0,1} then {2,3}) double-buffer load against compute.
  - Elimination updates run on groups of up to 4 planes per instruction
    (the plane index rides a third AP dim); each solution row is stored
    (TensorE transpose + DMA) as soon as its back-substitution step
    finishes, hiding the store under the remaining back pass.
"""

import os

import numpy as np

import concourse.bacc as bacc
import concourse.mybir as mybir
from concourse.bass_utils import run_bass_kernel_spmd
from concourse.masks import make_identity
from concourse.tile import TileContext

B, NRX, NR, U, A, S, F = 4, 1, 32, 4, 8, 14, 1024
NCORES = 8
FS = F // NCORES        # 128 subcarriers per core
NB = 2                  # batch entries per chunk
NCH = B // NB           # chunks per core
M = U * NB * S          # 112 RE columns per chunk (u, b, s)
NP = 9                  # augmented planes: 8 matrix columns + rhs
ND = 75                 # RE columns solved on DVE (rest go to Pool/GPSIMD)
F32 = mybir.dt.float32
AL = mybir.AluOpType

LAST_RESULTS = None     # BassKernelResults of the most recent run (for test.py)


def _build():
    nc = bacc.Bacc(trn_type="TRN2")

    # Host-prepped layouts, chosen so every per-(chunk, i) DMA slice is
    # stride-collapsible: hd[i, u, b, s, j, f], yd[i, u, b, s, f],
    # out[i, u, b, s, f, c].  (i = matrix row, j = matrix column.)
    hdre = nc.dram_tensor("hd_re", [A, U, B, S, A, FS], F32, kind="ExternalInput")
    hdim = nc.dram_tensor("hd_im", [A, U, B, S, A, FS], F32, kind="ExternalInput")
    ydre = nc.dram_tensor("yd_re", [A, U, B, S, FS], F32, kind="ExternalInput")
    ydim = nc.dram_tensor("yd_im", [A, U, B, S, FS], F32, kind="ExternalInput")
    out = nc.dram_tensor("out", [A, U, B, S, FS, 2], F32, kind="ExternalOutput")

    # (engine, column range) pairs: each engine owns cols [c0, c0+mw) of the
    # M RE columns and a private set of tiles sized to mw.
    def engines():
        return ((nc.vector, 0, ND), (nc.gpsimd, ND, M - ND))

    with TileContext(nc) as tc:
        with (
            tc.tile_pool(name="consts", bufs=1) as consts,
            tc.tile_pool(name="supers", bufs=2) as supers,
            tc.tile_pool(name="work", bufs=1) as work,
            tc.tile_pool(name="stg", bufs=2) as stg,
            tc.tile_pool(name="stgo", bufs=3) as stgo,
            tc.tile_pool(name="psin", bufs=3, space="PSUM") as psin,
            tc.tile_pool(name="psy", bufs=2, space="PSUM") as psy_pool,
            tc.tile_pool(name="pso", bufs=2, space="PSUM") as pso_pool,
        ):
            ident = consts.tile([128, 128], F32)
            make_identity(nc, ident)

            for ci in range(NCH):
                b0 = ci * NB
                # Per-engine supertiles: 10 planes (9 used + 1 pad for the
                # w-group AP views) x 8 rows x mw columns, re/im separate.
                sup = {}
                for eng, c0, mw in engines():
                    tag = f"H{c0}"
                    sup[c0] = (
                        supers.tile(
                            [128, (NP + 1) * A * mw], F32,
                            tag=tag + "re", name=tag + "re",
                        ),
                        supers.tile(
                            [128, (NP + 1) * A * mw], F32,
                            tag=tag + "im", name=tag + "im",
                        ),
                    )

                def off(j, i, mw):
                    return (j * A + i) * mw

                def row(T, j, i, mw):
                    return T[:, off(j, i, mw) : off(j, i, mw) + mw]

                def rows3(T, j, i0, n, mw):
                    base = off(j, i0, mw)
                    return T[:, base : base + n * mw].rearrange(
                        "p (r c) -> p r c", r=n
                    )

                # ---------------- load h ----------------
                for comp in range(2):
                    hsrc = (hdre, hdim)[comp]
                    for i in range(A):
                        stage = stg.tile([M, A * FS], F32, tag="stage")
                        src = hsrc[i, :, b0 : b0 + NB]
                        nc.sync.dma_start(stage, src)
                        for jg in range(2):
                            ps = psin.tile([128, 4 * M], F32, tag="psin")
                            for q in range(4):
                                j = jg * 4 + q
                                nc.tensor.transpose(
                                    ps[:, q * M : (q + 1) * M],
                                    stage[:, j * FS : (j + 1) * FS],
                                    ident[:M, :M],
                                )
                            src4 = ps.rearrange("p (q c) -> p q c", q=4)
                            for eng, c0, mw in engines():
                                base = off(jg * 4, i, mw)
                                dst = sup[c0][comp][
                                    :, base : base + 4 * A * mw
                                ].rearrange("p (q c) -> p q c", q=4)[:, :, :mw]
                                # chunk 1: the solve hasn't started, so idle
                                # DVE drains its own tile and Act covers the
                                # Pool columns (GPSIMD cannot read PSUM);
                                # chunk 2 overlaps the solve, so Act drains.
                                if ci == 0 and eng is nc.vector:
                                    eng.tensor_copy(dst, src4[:, :, c0 : c0 + mw])
                                else:
                                    nc.scalar.copy(dst, src4[:, :, c0 : c0 + mw])

                # ---------------- load y ----------------
                for comp in range(2):
                    ysrc = (ydre, ydim)[comp]
                    for i in range(A):
                        sy = stg.tile([M, FS], F32, tag="stagey")
                        nc.sync.dma_start(sy, ysrc[i, :, b0 : b0 + NB])
                        py = psy_pool.tile([128, M], F32, tag="psy")
                        nc.tensor.transpose(py, sy, ident[:M, :M])
                        for eng, c0, mw in engines():
                            nc.scalar.copy(
                                row(sup[c0][comp], 8, i, mw), py[:, c0 : c0 + mw]
                            )

                # ---------------- solve ----------------
                # Per-engine private work tiles.
                wt = {}
                for eng, c0, mw in engines():
                    tg = f"w{c0}"
                    sizes = dict(
                        INV=3 * A * mw, GRe=(A - 1) * mw, GIm=(A - 1) * mw,
                        PAs=4 * (A - 1) * mw, PBs=4 * (A - 1) * mw,
                        PCs=(A - 1) * mw, PDs=(A - 1) * mw,
                        TD=mw, TU=mw, TR=mw,
                    )
                    wt[c0] = {
                        nm: work.tile(
                            [128, sz], F32, tag=tg + nm, name=tg + nm
                        )
                        for nm, sz in sizes.items()
                    }

                def inv_pair(w, k, mw, n=None):
                    # (ir_k, ii_k) as [128, 2, mw]; broadcast over n rows
                    v = w["INV"][:, k * mw : k * mw + 2 * A * mw].rearrange(
                        "p (j c) -> p j c", j=2
                    )[:, :, :mw]
                    if n is None:
                        return v
                    return v[:, :, None, :].broadcast_to([128, 2, n, mw])

                # forward elimination
                for k in range(A):
                    # Pivot chain first, Pool's columns before DVE's own:
                    # |p|^2 + reciprocal run on Act + DVE for BOTH column
                    # blocks (Pool's ISA has no divide/reciprocal), and they
                    # must precede DVE's big update stream in DVE program
                    # order or Pool's step-k factors stall behind it.
                    for eng, c0, mw in reversed(engines()):
                        w = wt[c0]
                        HRe, HIm = sup[c0]
                        nc.scalar.square(w["TD"], row(HRe, k, k, mw))
                        nc.scalar.square(w["TU"], row(HIm, k, k, mw))
                        nc.vector.tensor_add(w["TD"], w["TD"], w["TU"])
                        nc.vector.reciprocal(w["TR"], w["TD"])
                    for eng, c0, mw in engines():
                        w = wt[c0]
                        HRe, HIm = sup[c0]
                        a = row(HRe, k, k, mw)
                        b_ = row(HIm, k, k, mw)
                        irk = w["INV"][:, k * mw : (k + 1) * mw]
                        iik = w["INV"][:, (A + k) * mw : (A + k + 1) * mw]
                        eng.tensor_mul(irk, a, w["TR"])
                        eng.tensor_mul(iik, b_, w["TR"])
                        n = A - 1 - k
                        if n == 0:
                            continue
                        # factors G = +H[i,k] * inv(p), via paired products:
                        #   PA = (a*ir || a*ii),  PB = (b*ir || b*ii)
                        car = rows3(HRe, k, k + 1, n, mw)
                        cai = rows3(HIm, k, k + 1, n, mw)
                        car4 = car[:, None, :, :].broadcast_to([128, 2, n, mw])
                        cai4 = cai[:, None, :, :].broadcast_to([128, 2, n, mw])

                        def sc4(T):
                            return T[:, : 2 * n * mw].rearrange(
                                "p (j r c) -> p j r c", j=2, r=n
                            )

                        def sc_half(T, h):
                            return T[:, h * n * mw : (h + 1) * n * mw]

                        eng.tensor_mul(sc4(w["PAs"]), car4, inv_pair(w, k, mw, n))
                        eng.tensor_mul(sc4(w["PBs"]), cai4, inv_pair(w, k, mw, n))
                        gre = w["GRe"][:, : n * mw]
                        gim = w["GIm"][:, : n * mw]
                        # gre = a*ir + b*ii, gim = b*ir - a*ii  (G = H[i,k]/p)
                        eng.tensor_add(gre, sc_half(w["PAs"], 0), sc_half(w["PBs"], 1))
                        eng.tensor_sub(gim, sc_half(w["PBs"], 0), sc_half(w["PAs"], 1))
                        # eliminate column k from planes k+1..7 and y, in
                        # groups of up to 4 planes per instruction: the plane
                        # index is a third AP dim (stride A*mw), so one
                        # [128, wg, n, mw] op covers wg planes.  H[i,j] -= G*B
                        # (complex), with products cycling through PAs/PBs.
                        js = list(range(k + 1, NP))
                        while js:
                            wg = min(4, len(js))
                            j0 = js[0]
                            js = js[wg:]

                            def wrows(T):
                                base = off(j0, k + 1, mw)
                                return T[:, base : base + wg * A * mw].rearrange(
                                    "p (w c) -> p w c", w=wg
                                )[:, :, : n * mw]

                            def wrow_b(T):
                                base = off(j0, k, mw)
                                v = T[:, base : base + wg * A * mw].rearrange(
                                    "p (w c) -> p w c", w=wg
                                )[:, :, :mw]
                                return v[:, :, None, :].broadcast_to(
                                    [128, wg, n, mw]
                                )

                            def fw(Ft):
                                v = Ft[:, : n * mw].rearrange(
                                    "p (r c) -> p r c", r=n
                                )
                                return v[:, None, :, :].broadcast_to(
                                    [128, wg, n, mw]
                                )

                            hr, hi = wrows(HRe), wrows(HIm)
                            Br, Bi = wrow_b(HRe), wrow_b(HIm)
                            grew, gimw = fw(w["GRe"]), fw(w["GIm"])
                            SA4 = w["PAs"][:, : wg * n * mw].rearrange(
                                "p (w r c) -> p w r c", w=wg, r=n
                            )
                            SA3 = w["PAs"][:, : wg * n * mw].rearrange(
                                "p (w c) -> p w c", w=wg
                            )
                            SB4 = w["PBs"][:, : wg * n * mw].rearrange(
                                "p (w r c) -> p w r c", w=wg, r=n
                            )
                            SB3 = w["PBs"][:, : wg * n * mw].rearrange(
                                "p (w c) -> p w c", w=wg
                            )
                            # H[i,j] -= G*B (complex); products regrouped by
                            # factor so consecutive ops never share a RAW
                            # destination (longer dep gaps -> less ack stall)
                            eng.tensor_mul(SA4, grew, Br)
                            eng.tensor_mul(SB4, grew, Bi)
                            eng.tensor_sub(hr, hr, SA3)
                            eng.tensor_sub(hi, hi, SB3)
                            eng.tensor_mul(SA4, gimw, Bi)
                            eng.tensor_mul(SB4, gimw, Br)
                            eng.tensor_add(hr, hr, SA3)
                            eng.tensor_sub(hi, hi, SB3)

                # back substitution (Jordan): x_k = y_k*invp, then clear col k
                for k in range(A - 1, -1, -1):
                    # xrow holds x_k full-width (re || im) so a single PE
                    # transpose per component can stage the store; bufs=2
                    # decouples consecutive k stores.
                    xrow = stgo.tile([128, 2 * M], F32, tag="xrow", name="xrow")
                    for eng, c0, mw in engines():
                        w = wt[c0]
                        HRe, HIm = sup[c0]
                        yr = row(HRe, 8, k, mw)
                        yi = row(HIm, 8, k, mw)
                        # p1 = (yr*ir || yr*ii), p2 = (yi*ir || yi*ii)
                        p1 = w["PAs"][:, : 2 * mw].rearrange("p (j c) -> p j c", j=2)
                        p2 = w["PBs"][:, : 2 * mw].rearrange("p (j c) -> p j c", j=2)
                        yr2 = yr[:, None, :].broadcast_to([128, 2, mw])
                        yi2 = yi[:, None, :].broadcast_to([128, 2, mw])
                        eng.tensor_mul(p1, yr2, inv_pair(w, k, mw))
                        eng.tensor_mul(p2, yi2, inv_pair(w, k, mw))
                        # x = y*conj(p)/|p|^2: xr = yr*ir + yi*ii,
                        #                      xi = yi*ir - yr*ii
                        eng.tensor_add(
                            xrow[:, c0 : c0 + mw],
                            w["PAs"][:, :mw], w["PBs"][:, mw : 2 * mw],
                        )
                        eng.tensor_sub(
                            xrow[:, M + c0 : M + c0 + mw],
                            w["PBs"][:, :mw], w["PAs"][:, mw : 2 * mw],
                        )
                    # x_k is final now -- store it while the rest of the back
                    # pass still runs on the elementwise engines.
                    so = stgo.tile([M, 2 * FS], F32, tag="so")
                    so3 = so.rearrange("p (f c) -> p f c", c=2)
                    for comp in range(2):
                        po = pso_pool.tile([M, FS], F32, tag="pso")
                        nc.tensor.transpose(
                            po, xrow[:, comp * M : (comp + 1) * M],
                            ident[:128, :128],
                        )
                        nc.scalar.copy(so3[:, :, comp], po)
                    dst = out[k, :, b0 : b0 + NB]
                    nc.sync.dma_start(dst, so)
                    if k == 0:
                        continue
                    for eng, c0, mw in engines():
                        w = wt[c0]
                        HRe, HIm = sup[c0]
                        xr_p = xrow[:, c0 : c0 + mw]
                        xi_p = xrow[:, M + c0 : M + c0 + mw]
                        cr = rows3(HRe, k, 0, k, mw)
                        ci_ = rows3(HIm, k, 0, k, mw)
                        xrB = xr_p[:, None, :].broadcast_to([128, k, mw])
                        xiB = xi_p[:, None, :].broadcast_to([128, k, mw])

                        def sc3(T):
                            return T[:, : k * mw].rearrange("p (r c) -> p r c", r=k)

                        qa, qb, qc, qd = (
                            sc3(w[t]) for t in ("PAs", "PBs", "PCs", "PDs")
                        )
                        eng.tensor_mul(qa, cr, xrB)
                        eng.tensor_mul(qb, ci_, xiB)
                        eng.tensor_mul(qc, cr, xiB)
                        eng.tensor_mul(qd, ci_, xrB)
                        ytr = rows3(HRe, 8, 0, k, mw)
                        yti = rows3(HIm, 8, 0, k, mw)
                        # y_i -= H[i,k] * x_k
                        eng.tensor_sub(ytr, ytr, qa)
                        eng.tensor_add(ytr, ytr, qb)
                        eng.tensor_sub(yti, yti, qc)
                        eng.tensor_sub(yti, yti, qd)

    nc.finalize()
    return nc


_NC_CACHE = None


def _get_nc():
    global _NC_CACHE
    if _NC_CACHE is None:
        _NC_CACHE = _build()
    return _NC_CACHE


def _prep_core(y_re, y_im, h_re, h_im, c):
    """Host-side shard prep for core c: f-slice + block-diagonal extraction."""
    fsl = slice(c * FS, (c + 1) * FS)
    ue = np.arange(U)
    maps = {}
    for name, h in (("hd_re", h_re), ("hd_im", h_im)):
        h6 = h[:, 0, :, :, :, :, fsl].reshape(B, U, A, U, A, S, FS)
        hd = h6[:, ue, :, ue]              # [u, b, i, j, s, f]
        maps[name] = np.ascontiguousarray(
            hd.transpose(2, 0, 1, 4, 3, 5), dtype=np.float32
        )                                   # [i, u, b, s, j, f]
    for name, y in (("yd_re", y_re), ("yd_im", y_im)):
        y5 = y[:, 0, :, :, fsl].reshape(B, U, A, S, FS)   # [b, u, i, s, f]
        maps[name] = np.ascontiguousarray(
            y5.transpose(2, 1, 0, 3, 4), dtype=np.float32
        )                                   # [i, u, b, s, f]
    return maps


def kernel(y_re, y_im, h_re, h_im, **_ignored):
    global LAST_RESULTS
    y_re = np.asarray(y_re, dtype=np.float32)
    y_im = np.asarray(y_im, dtype=np.float32)
    h_re = np.asarray(h_re, dtype=np.float32)
    h_im = np.asarray(h_im, dtype=np.float32)

    nc = _get_nc()
    in_maps = [_prep_core(y_re, y_im, h_re, h_im, c) for c in range(NCORES)]
    trace = bool(int(os.environ.get("BD_TRACE", "0")))
    res = run_bass_kernel_spmd(
        nc, in_maps, core_ids=list(range(NCORES)), trace=trace
    )
    LAST_RESULTS = res
    outs = []
    for r in res.results:
        o = r["out"]                              # [i, u, b, s, f, c]
        o = o.transpose(2, 1, 0, 3, 4, 5)         # [b, u, i, s, f, c]
        outs.append(o.reshape(B, NR, S, FS, 2))
    full = np.concatenate(outs, axis=3)           # [B, NR, S, F, 2]
    return np.ascontiguousarray(full[:, None])    # [B, 1, NR, S, F, 2]


# revision 15
# speedup vs baseline: 1.0418x; 1.0418x over previous
"""Block-diagonal ZF equalizer (nn_BDEqualizer) as a Trainium2 Bass kernel.

Math: for every resource element (b, s, f) and UE u, solve the 8x8 complex
system H_u x_u = y_u where H_u[i, j] = h[b, 0, 8u+i, u, j, s, f] and
y_u[i] = y[b, 0, 8u+i, s, f].  Output x as [B, 1, 32, S, F, 2] (re/im last).

Strategy (data-parallel over the fft axis, per the sharding hint):
  - 8 cores, each owns a contiguous 128-subcarrier slice of F=1024.
  - Host pre-extracts the block-diagonal channel blocks (pure indexing) and
    ships per-core shards hd[B, U, 8, 8, S, 128] / yd[B, U, 8, S, 128].
  - On-chip layout: subcarriers on the 128 SBUF partitions, the other RE
    axes (u, b-pair, s) = 112 along the free dim.  Each of the 9 augmented
    matrix columns (8 of H + rhs) is a "plane" of 8 rows; every Gaussian
    elimination step is a full-width elementwise op, with per-RE pivot
    reciprocals.  Unpivoted LU + Jordan back-substitution, complex
    arithmetic as separate re/im tiles.
  - The 112 RE columns are split across TWO elementwise engines that run
    the whole solve independently on disjoint column blocks held in
    separate supertiles: DVE (~1.04 ns/elem fp32) takes ND columns and
    Pool/GPSIMD (~1.98 ns/elem via TensorTensor) takes the rest.  Pool has
    no reciprocal or scalar_tensor_tensor, so pivot reciprocals use a
    ones/x TensorTensor divide and factors are computed sign-positive
    (G = +H[i,k]*inv(p)) so only plain add/sub/mult TT ops are needed.
  - TensorE transposes move between the DMA-friendly [(u,b,s), f] staging
    layout and the compute layout [f, (u,b,s)]; ScalarE drains PSUM into
    the per-engine supertiles and computes the pivot |p|^2 squares.
  - Two chunks (b in {0,1} then {2,3}) double-buffer load against compute.
  - Elimination updates run on groups of up to 4 planes per instruction
    (the plane index rides a third AP dim); each solution row is stored
    (TensorE transpose + DMA) as soon as its back-substitution step
    finishes, hiding the store under the remaining back pass.
"""

import os

import numpy as np

import concourse.bacc as bacc
import concourse.mybir as mybir
from concourse.bass_utils import run_bass_kernel_spmd
from concourse.masks import make_identity
from concourse.tile import TileContext

B, NRX, NR, U, A, S, F = 4, 1, 32, 4, 8, 14, 1024
NCORES = 8
FS = F // NCORES        # 128 subcarriers per core
NB = 2                  # batch entries per chunk
NCH = B // NB           # chunks per core
M = U * NB * S          # 112 RE columns per chunk (u, b, s)
NP = 9                  # augmented planes: 8 matrix columns + rhs
ND = 75                 # RE columns solved on DVE (rest go to Pool/GPSIMD)
F32 = mybir.dt.float32
AL = mybir.AluOpType

LAST_RESULTS = None     # BassKernelResults of the most recent run (for test.py)


def _build():
    nc = bacc.Bacc(trn_type="TRN2")

    # Host-prepped layouts, chosen so every per-(chunk, i) DMA slice is
    # stride-collapsible: hd[i, u, b, s, j, f], yd[i, u, b, s, f],
    # out[i, u, b, s, f, c].  (i = matrix row, j = matrix column.)
    # h shards are plane(j)-major so elimination step 0 can start once the
    # first few planes have landed, hiding part of the chunk-1 DMA head.
    hdre = nc.dram_tensor("hd_re", [A, U, B, S, A, FS], F32, kind="ExternalInput")
    hdim = nc.dram_tensor("hd_im", [A, U, B, S, A, FS], F32, kind="ExternalInput")
    ydre = nc.dram_tensor("yd_re", [A, U, B, S, FS], F32, kind="ExternalInput")
    ydim = nc.dram_tensor("yd_im", [A, U, B, S, FS], F32, kind="ExternalInput")
    out = nc.dram_tensor("out", [A, U, B, S, FS, 2], F32, kind="ExternalOutput")

    # (engine, column range) pairs: each engine owns cols [c0, c0+mw) of the
    # M RE columns and a private set of tiles sized to mw.
    def engines():
        return ((nc.vector, 0, ND), (nc.gpsimd, ND, M - ND))

    with TileContext(nc) as tc:
        with (
            tc.tile_pool(name="consts", bufs=1) as consts,
            tc.tile_pool(name="supers", bufs=2) as supers,
            tc.tile_pool(name="work", bufs=1) as work,
            tc.tile_pool(name="stg", bufs=2) as stg,
            tc.tile_pool(name="stgo", bufs=3) as stgo,
            tc.tile_pool(name="psin", bufs=3, space="PSUM") as psin,
            tc.tile_pool(name="psy", bufs=2, space="PSUM") as psy_pool,
            tc.tile_pool(name="pso", bufs=2, space="PSUM") as pso_pool,
        ):
            ident = consts.tile([128, 128], F32)
            make_identity(nc, ident)

            for ci in range(NCH):
                b0 = ci * NB
                # Per-engine supertiles: 10 planes (9 used + 1 pad for the
                # w-group AP views) x 8 rows x mw columns, re/im separate.
                sup = {}
                for eng, c0, mw in engines():
                    tag = f"H{c0}"
                    sup[c0] = (
                        supers.tile(
                            [128, (NP + 1) * A * mw], F32,
                            tag=tag + "re", name=tag + "re",
                        ),
                        supers.tile(
                            [128, (NP + 1) * A * mw], F32,
                            tag=tag + "im", name=tag + "im",
                        ),
                    )

                def off(j, i, mw):
                    return (j * A + i) * mw

                def row(T, j, i, mw):
                    return T[:, off(j, i, mw) : off(j, i, mw) + mw]

                def rows3(T, j, i0, n, mw):
                    base = off(j, i0, mw)
                    return T[:, base : base + n * mw].rearrange(
                        "p (r c) -> p r c", r=n
                    )

                # ---------------- load h (plane-major) ----------------
                for j in range(A):
                    for comp in range(2):
                        hsrc = (hdre, hdim)[comp]
                        stage = stg.tile([M, A * FS], F32, tag="stage")
                        src = hsrc[j, :, b0 : b0 + NB]
                        nc.sync.dma_start(stage, src)
                        for ig in range(2):
                            ps = psin.tile([128, 4 * M], F32, tag="psin")
                            for q in range(4):
                                i = ig * 4 + q
                                nc.tensor.transpose(
                                    ps[:, q * M : (q + 1) * M],
                                    stage[:, i * FS : (i + 1) * FS],
                                    ident[:M, :M],
                                )
                            src4 = ps.rearrange("p (q c) -> p q c", q=4)
                            for eng, c0, mw in engines():
                                base = off(j, ig * 4, mw)
                                dst = sup[c0][comp][
                                    :, base : base + 4 * mw
                                ].rearrange("p (q c) -> p q c", q=4)
                                nc.scalar.copy(dst, src4[:, :, c0 : c0 + mw])

                # ---------------- load y ----------------
                for comp in range(2):
                    ysrc = (ydre, ydim)[comp]
                    for i in range(A):
                        sy = stg.tile([M, FS], F32, tag="stagey")
                        nc.sync.dma_start(sy, ysrc[i, :, b0 : b0 + NB])
                        py = psy_pool.tile([128, M], F32, tag="psy")
                        nc.tensor.transpose(py, sy, ident[:M, :M])
                        for eng, c0, mw in engines():
                            nc.scalar.copy(
                                row(sup[c0][comp], 8, i, mw), py[:, c0 : c0 + mw]
                            )

                # ---------------- solve ----------------
                # Per-engine private work tiles.
                wt = {}
                for eng, c0, mw in engines():
                    tg = f"w{c0}"
                    sizes = dict(
                        INV=3 * A * mw, GRe=(A - 1) * mw, GIm=(A - 1) * mw,
                        PAs=4 * (A - 1) * mw, PBs=4 * (A - 1) * mw,
                        PCs=(A - 1) * mw, PDs=(A - 1) * mw,
                        TD=mw, TU=mw, TR=mw,
                    )
                    wt[c0] = {
                        nm: work.tile(
                            [128, sz], F32, tag=tg + nm, name=tg + nm
                        )
                        for nm, sz in sizes.items()
                    }

                def inv_pair(w, k, mw, n=None):
                    # (ir_k, ii_k) as [128, 2, mw]; broadcast over n rows
                    v = w["INV"][:, k * mw : k * mw + 2 * A * mw].rearrange(
                        "p (j c) -> p j c", j=2
                    )[:, :, :mw]
                    if n is None:
                        return v
                    return v[:, :, None, :].broadcast_to([128, 2, n, mw])

                # forward elimination
                for k in range(A):
                    # Pivot chain first, Pool's columns before DVE's own:
                    # |p|^2 + reciprocal run on Act + DVE for BOTH column
                    # blocks (Pool's ISA has no divide/reciprocal), and they
                    # must precede DVE's big update stream in DVE program
                    # order or Pool's step-k factors stall behind it.
                    # Interleave the per-column-block ops so consecutive DVE
                    # instructions never form a RAW pair (hides the ~95ns
                    # SBUF write-ack latency between dependent small ops).
                    for eng, c0, mw in reversed(engines()):
                        w = wt[c0]
                        HRe, HIm = sup[c0]
                        nc.scalar.square(w["TD"], row(HRe, k, k, mw))
                        nc.scalar.square(w["TU"], row(HIm, k, k, mw))
                    for eng, c0, mw in reversed(engines()):
                        w = wt[c0]
                        nc.vector.tensor_add(w["TD"], w["TD"], w["TU"])
                    for eng, c0, mw in reversed(engines()):
                        w = wt[c0]
                        nc.vector.reciprocal(w["TR"], w["TD"])
                    for eng, c0, mw in engines():
                        w = wt[c0]
                        HRe, HIm = sup[c0]
                        a = row(HRe, k, k, mw)
                        b_ = row(HIm, k, k, mw)
                        irk = w["INV"][:, k * mw : (k + 1) * mw]
                        iik = w["INV"][:, (A + k) * mw : (A + k + 1) * mw]
                        eng.tensor_mul(irk, a, w["TR"])
                        eng.tensor_mul(iik, b_, w["TR"])
                        n = A - 1 - k
                        if n == 0:
                            continue
                        # factors G = +H[i,k] * inv(p), via paired products:
                        #   PA = (a*ir || a*ii),  PB = (b*ir || b*ii)
                        car = rows3(HRe, k, k + 1, n, mw)
                        cai = rows3(HIm, k, k + 1, n, mw)
                        car4 = car[:, None, :, :].broadcast_to([128, 2, n, mw])
                        cai4 = cai[:, None, :, :].broadcast_to([128, 2, n, mw])

                        def sc4(T):
                            return T[:, : 2 * n * mw].rearrange(
                                "p (j r c) -> p j r c", j=2, r=n
                            )

                        def sc_half(T, h):
                            return T[:, h * n * mw : (h + 1) * n * mw]

                        eng.tensor_mul(sc4(w["PAs"]), car4, inv_pair(w, k, mw, n))
                        eng.tensor_mul(sc4(w["PBs"]), cai4, inv_pair(w, k, mw, n))
                        gre = w["GRe"][:, : n * mw]
                        gim = w["GIm"][:, : n * mw]
                        # gre = a*ir + b*ii, gim = b*ir - a*ii  (G = H[i,k]/p)
                        eng.tensor_add(gre, sc_half(w["PAs"], 0), sc_half(w["PBs"], 1))
                        eng.tensor_sub(gim, sc_half(w["PBs"], 0), sc_half(w["PAs"], 1))
                        # eliminate column k from planes k+1..7 and y, in
                        # groups of up to 4 planes per instruction: the plane
                        # index is a third AP dim (stride A*mw), so one
                        # [128, wg, n, mw] op covers wg planes.  H[i,j] -= G*B
                        # (complex), with products cycling through PAs/PBs.
                        js = list(range(k + 1, NP))
                        while js:
                            wg = min(4, len(js))
                            j0 = js[0]
                            js = js[wg:]

                            def wrows(T):
                                base = off(j0, k + 1, mw)
                                return T[:, base : base + wg * A * mw].rearrange(
                                    "p (w c) -> p w c", w=wg
                                )[:, :, : n * mw]

                            def wrow_b(T):
                                base = off(j0, k, mw)
                                v = T[:, base : base + wg * A * mw].rearrange(
                                    "p (w c) -> p w c", w=wg
                                )[:, :, :mw]
                                return v[:, :, None, :].broadcast_to(
                                    [128, wg, n, mw]
                                )

                            def fw(Ft):
                                v = Ft[:, : n * mw].rearrange(
                                    "p (r c) -> p r c", r=n
                                )
                                return v[:, None, :, :].broadcast_to(
                                    [128, wg, n, mw]
                                )

                            hr, hi = wrows(HRe), wrows(HIm)
                            Br, Bi = wrow_b(HRe), wrow_b(HIm)
                            grew, gimw = fw(w["GRe"]), fw(w["GIm"])
                            SA4 = w["PAs"][:, : wg * n * mw].rearrange(
                                "p (w r c) -> p w r c", w=wg, r=n
                            )
                            SA3 = w["PAs"][:, : wg * n * mw].rearrange(
                                "p (w c) -> p w c", w=wg
                            )
                            SB4 = w["PBs"][:, : wg * n * mw].rearrange(
                                "p (w r c) -> p w r c", w=wg, r=n
                            )
                            SB3 = w["PBs"][:, : wg * n * mw].rearrange(
                                "p (w c) -> p w c", w=wg
                            )
                            # H[i,j] -= G*B (complex); products regrouped by
                            # factor so consecutive ops never share a RAW
                            # destination (longer dep gaps -> less ack stall)
                            eng.tensor_mul(SA4, grew, Br)
                            eng.tensor_mul(SB4, grew, Bi)
                            eng.tensor_sub(hr, hr, SA3)
                            eng.tensor_sub(hi, hi, SB3)
                            eng.tensor_mul(SA4, gimw, Bi)
                            eng.tensor_mul(SB4, gimw, Br)
                            eng.tensor_add(hr, hr, SA3)
                            eng.tensor_sub(hi, hi, SB3)

                # back substitution (Jordan): x_k = y_k*invp, then clear col k
                for k in range(A - 1, -1, -1):
                    # xrow holds x_k full-width (re || im) so a single PE
                    # transpose per component can stage the store; bufs=2
                    # decouples consecutive k stores.
                    xrow = stgo.tile([128, 2 * M], F32, tag="xrow", name="xrow")
                    for eng, c0, mw in engines():
                        w = wt[c0]
                        HRe, HIm = sup[c0]
                        yr = row(HRe, 8, k, mw)
                        yi = row(HIm, 8, k, mw)
                        # p1 = (yr*ir || yr*ii), p2 = (yi*ir || yi*ii)
                        p1 = w["PAs"][:, : 2 * mw].rearrange("p (j c) -> p j c", j=2)
                        p2 = w["PBs"][:, : 2 * mw].rearrange("p (j c) -> p j c", j=2)
                        yr2 = yr[:, None, :].broadcast_to([128, 2, mw])
                        yi2 = yi[:, None, :].broadcast_to([128, 2, mw])
                        eng.tensor_mul(p1, yr2, inv_pair(w, k, mw))
                        eng.tensor_mul(p2, yi2, inv_pair(w, k, mw))
                        # x = y*conj(p)/|p|^2: xr = yr*ir + yi*ii,
                        #                      xi = yi*ir - yr*ii
                        eng.tensor_add(
                            xrow[:, c0 : c0 + mw],
                            w["PAs"][:, :mw], w["PBs"][:, mw : 2 * mw],
                        )
                        eng.tensor_sub(
                            xrow[:, M + c0 : M + c0 + mw],
                            w["PBs"][:, :mw], w["PAs"][:, mw : 2 * mw],
                        )
                    # x_k is final now -- store it while the rest of the back
                    # pass still runs on the elementwise engines.
                    so = stgo.tile([M, 2 * FS], F32, tag="so")
                    so3 = so.rearrange("p (f c) -> p f c", c=2)
                    for comp in range(2):
                        po = pso_pool.tile([M, FS], F32, tag="pso")
                        nc.tensor.transpose(
                            po, xrow[:, comp * M : (comp + 1) * M],
                            ident[:128, :128],
                        )
                        nc.scalar.copy(so3[:, :, comp], po)
                    dst = out[k, :, b0 : b0 + NB]
                    nc.sync.dma_start(dst, so)
                    if k == 0:
                        continue
                    for eng, c0, mw in engines():
                        w = wt[c0]
                        HRe, HIm = sup[c0]
                        xr_p = xrow[:, c0 : c0 + mw]
                        xi_p = xrow[:, M + c0 : M + c0 + mw]
                        cr = rows3(HRe, k, 0, k, mw)
                        ci_ = rows3(HIm, k, 0, k, mw)
                        xrB = xr_p[:, None, :].broadcast_to([128, k, mw])
                        xiB = xi_p[:, None, :].broadcast_to([128, k, mw])

                        def sc3(T):
                            return T[:, : k * mw].rearrange("p (r c) -> p r c", r=k)

                        qa, qb, qc, qd = (
                            sc3(w[t]) for t in ("PAs", "PBs", "PCs", "PDs")
                        )
                        eng.tensor_mul(qa, cr, xrB)
                        eng.tensor_mul(qb, ci_, xiB)
                        eng.tensor_mul(qc, cr, xiB)
                        eng.tensor_mul(qd, ci_, xrB)
                        ytr = rows3(HRe, 8, 0, k, mw)
                        yti = rows3(HIm, 8, 0, k, mw)
                        # y_i -= H[i,k] * x_k
                        eng.tensor_sub(ytr, ytr, qa)
                        eng.tensor_add(ytr, ytr, qb)
                        eng.tensor_sub(yti, yti, qc)
                        eng.tensor_sub(yti, yti, qd)

    nc.finalize()
    return nc


_NC_CACHE = None


def _get_nc():
    global _NC_CACHE
    if _NC_CACHE is None:
        _NC_CACHE = _build()
    return _NC_CACHE


def _prep_core(y_re, y_im, h_re, h_im, c):
    """Host-side shard prep for core c: f-slice + block-diagonal extraction."""
    fsl = slice(c * FS, (c + 1) * FS)
    ue = np.arange(U)
    maps = {}
    for name, h in (("hd_re", h_re), ("hd_im", h_im)):
        h6 = h[:, 0, :, :, :, :, fsl].reshape(B, U, A, U, A, S, FS)
        hd = h6[:, ue, :, ue]              # [u, b, i, j, s, f]
        maps[name] = np.ascontiguousarray(
            hd.transpose(3, 0, 1, 4, 2, 5), dtype=np.float32
        )                                   # [j, u, b, s, i, f]
    for name, y in (("yd_re", y_re), ("yd_im", y_im)):
        y5 = y[:, 0, :, :, fsl].reshape(B, U, A, S, FS)   # [b, u, i, s, f]
        maps[name] = np.ascontiguousarray(
            y5.transpose(2, 1, 0, 3, 4), dtype=np.float32
        )                                   # [i, u, b, s, f]
    return maps


def kernel(y_re, y_im, h_re, h_im, **_ignored):
    global LAST_RESULTS
    y_re = np.asarray(y_re, dtype=np.float32)
    y_im = np.asarray(y_im, dtype=np.float32)
    h_re = np.asarray(h_re, dtype=np.float32)
    h_im = np.asarray(h_im, dtype=np.float32)

    nc = _get_nc()
    in_maps = [_prep_core(y_re, y_im, h_re, h_im, c) for c in range(NCORES)]
    trace = bool(int(os.environ.get("BD_TRACE", "0")))
    res = run_bass_kernel_spmd(
        nc, in_maps, core_ids=list(range(NCORES)), trace=trace
    )
    LAST_RESULTS = res
    outs = []
    for r in res.results:
        o = r["out"]                              # [i, u, b, s, f, c]
        o = o.transpose(2, 1, 0, 3, 4, 5)         # [b, u, i, s, f, c]
        outs.append(o.reshape(B, NR, S, FS, 2))
    full = np.concatenate(outs, axis=3)           # [B, NR, S, F, 2]
    return np.ascontiguousarray(full[:, None])    # [B, 1, NR, S, F, 2]


# revision 16
# speedup vs baseline: 1.0451x; 1.0032x over previous
"""Block-diagonal ZF equalizer (nn_BDEqualizer) as a Trainium2 Bass kernel.

Math: for every resource element (b, s, f) and UE u, solve the 8x8 complex
system H_u x_u = y_u where H_u[i, j] = h[b, 0, 8u+i, u, j, s, f] and
y_u[i] = y[b, 0, 8u+i, s, f].  Output x as [B, 1, 32, S, F, 2] (re/im last).

Strategy (data-parallel over the fft axis, per the sharding hint):
  - 8 cores, each owns a contiguous 128-subcarrier slice of F=1024.
  - Host pre-extracts the block-diagonal channel blocks (pure indexing) and
    ships per-core shards hd[B, U, 8, 8, S, 128] / yd[B, U, 8, S, 128].
  - On-chip layout: subcarriers on the 128 SBUF partitions, the other RE
    axes (u, b-pair, s) = 112 along the free dim.  Each of the 9 augmented
    matrix columns (8 of H + rhs) is a "plane" of 8 rows; every Gaussian
    elimination step is a full-width elementwise op, with per-RE pivot
    reciprocals.  Unpivoted LU + Jordan back-substitution, complex
    arithmetic as separate re/im tiles.
  - The 112 RE columns are split across TWO elementwise engines that run
    the whole solve independently on disjoint column blocks held in
    separate supertiles: DVE (~1.04 ns/elem fp32) takes ND columns and
    Pool/GPSIMD (~1.98 ns/elem via TensorTensor) takes the rest.  Pool has
    no reciprocal or scalar_tensor_tensor, so pivot reciprocals use a
    ones/x TensorTensor divide and factors are computed sign-positive
    (G = +H[i,k]*inv(p)) so only plain add/sub/mult TT ops are needed.
  - TensorE transposes move between the DMA-friendly [(u,b,s), f] staging
    layout and the compute layout [f, (u,b,s)]; ScalarE drains PSUM into
    the per-engine supertiles and computes the pivot |p|^2 squares.
  - Two chunks (b in {0,1} then {2,3}) double-buffer load against compute.
  - Elimination updates run on groups of up to 4 planes per instruction
    (the plane index rides a third AP dim); each solution row is stored
    (TensorE transpose + DMA) as soon as its back-substitution step
    finishes, hiding the store under the remaining back pass.
"""

import os

import numpy as np

import concourse.bacc as bacc
import concourse.mybir as mybir
from concourse.bass_utils import run_bass_kernel_spmd
from concourse.masks import make_identity
from concourse.tile import TileContext

B, NRX, NR, U, A, S, F = 4, 1, 32, 4, 8, 14, 1024
NCORES = 8
FS = F // NCORES        # 128 subcarriers per core
NB = 2                  # batch entries per chunk
NCH = B // NB           # chunks per core
M = U * NB * S          # 112 RE columns per chunk (u, b, s)
NP = 9                  # augmented planes: 8 matrix columns + rhs
ND = 74                 # RE columns solved on DVE (rest go to Pool/GPSIMD)
F32 = mybir.dt.float32
AL = mybir.AluOpType

LAST_RESULTS = None     # BassKernelResults of the most recent run (for test.py)


def _build():
    nc = bacc.Bacc(trn_type="TRN2")

    # Host-prepped layouts, chosen so every per-(chunk, i) DMA slice is
    # stride-collapsible: hd[i, u, b, s, j, f], yd[i, u, b, s, f],
    # out[i, u, b, s, f, c].  (i = matrix row, j = matrix column.)
    # h shards are plane(j)-major so elimination step 0 can start once the
    # first few planes have landed, hiding part of the chunk-1 DMA head.
    hdre = nc.dram_tensor("hd_re", [A, U, B, S, A, FS], F32, kind="ExternalInput")
    hdim = nc.dram_tensor("hd_im", [A, U, B, S, A, FS], F32, kind="ExternalInput")
    ydre = nc.dram_tensor("yd_re", [A, U, B, S, FS], F32, kind="ExternalInput")
    ydim = nc.dram_tensor("yd_im", [A, U, B, S, FS], F32, kind="ExternalInput")
    out = nc.dram_tensor("out", [A, U, B, S, FS, 2], F32, kind="ExternalOutput")

    # (engine, column range) pairs: each engine owns cols [c0, c0+mw) of the
    # M RE columns and a private set of tiles sized to mw.
    def engines():
        return ((nc.vector, 0, ND), (nc.gpsimd, ND, M - ND))

    with TileContext(nc) as tc:
        with (
            tc.tile_pool(name="consts", bufs=1) as consts,
            tc.tile_pool(name="supers", bufs=2) as supers,
            tc.tile_pool(name="work", bufs=1) as work,
            tc.tile_pool(name="stg", bufs=2) as stg,
            tc.tile_pool(name="stgo", bufs=3) as stgo,
            tc.tile_pool(name="psin", bufs=3, space="PSUM") as psin,
            tc.tile_pool(name="psy", bufs=2, space="PSUM") as psy_pool,
            tc.tile_pool(name="pso", bufs=2, space="PSUM") as pso_pool,
        ):
            ident = consts.tile([128, 128], F32)
            make_identity(nc, ident)

            for ci in range(NCH):
                b0 = ci * NB
                # Per-engine supertiles: 10 planes (9 used + 1 pad for the
                # w-group AP views) x 8 rows x mw columns, re/im separate.
                sup = {}
                for eng, c0, mw in engines():
                    tag = f"H{c0}"
                    sup[c0] = (
                        supers.tile(
                            [128, (NP + 1) * A * mw], F32,
                            tag=tag + "re", name=tag + "re",
                        ),
                        supers.tile(
                            [128, (NP + 1) * A * mw], F32,
                            tag=tag + "im", name=tag + "im",
                        ),
                    )

                def off(j, i, mw):
                    return (j * A + i) * mw

                def row(T, j, i, mw):
                    return T[:, off(j, i, mw) : off(j, i, mw) + mw]

                def rows3(T, j, i0, n, mw):
                    base = off(j, i0, mw)
                    return T[:, base : base + n * mw].rearrange(
                        "p (r c) -> p r c", r=n
                    )

                # ---------------- load h (plane-major) ----------------
                for j in range(A):
                    for comp in range(2):
                        hsrc = (hdre, hdim)[comp]
                        stage = stg.tile([M, A * FS], F32, tag="stage")
                        src = hsrc[j, :, b0 : b0 + NB]
                        nc.sync.dma_start(stage, src)
                        for ig in range(2):
                            ps = psin.tile([128, 4 * M], F32, tag="psin")
                            for q in range(4):
                                i = ig * 4 + q
                                nc.tensor.transpose(
                                    ps[:, q * M : (q + 1) * M],
                                    stage[:, i * FS : (i + 1) * FS],
                                    ident[:M, :M],
                                )
                            src4 = ps.rearrange("p (q c) -> p q c", q=4)
                            for eng, c0, mw in engines():
                                base = off(j, ig * 4, mw)
                                dst = sup[c0][comp][
                                    :, base : base + 4 * mw
                                ].rearrange("p (q c) -> p q c", q=4)
                                nc.scalar.copy(dst, src4[:, :, c0 : c0 + mw])

                # ---------------- load y ----------------
                for comp in range(2):
                    ysrc = (ydre, ydim)[comp]
                    for i in range(A):
                        sy = stg.tile([M, FS], F32, tag="stagey")
                        nc.sync.dma_start(sy, ysrc[i, :, b0 : b0 + NB])
                        py = psy_pool.tile([128, M], F32, tag="psy")
                        nc.tensor.transpose(py, sy, ident[:M, :M])
                        for eng, c0, mw in engines():
                            nc.scalar.copy(
                                row(sup[c0][comp], 8, i, mw), py[:, c0 : c0 + mw]
                            )

                # ---------------- solve ----------------
                # Per-engine private work tiles.
                wt = {}
                for eng, c0, mw in engines():
                    tg = f"w{c0}"
                    sizes = dict(
                        INV=3 * A * mw, GRe=(A - 1) * mw, GIm=(A - 1) * mw,
                        PAs=4 * (A - 1) * mw, PBs=4 * (A - 1) * mw,
                        PCs=(A - 1) * mw, PDs=(A - 1) * mw,
                        TD=mw, TU=mw, TR=mw,
                    )
                    wt[c0] = {
                        nm: work.tile(
                            [128, sz], F32, tag=tg + nm, name=tg + nm
                        )
                        for nm, sz in sizes.items()
                    }

                def inv_pair(w, k, mw, n=None):
                    # (ir_k, ii_k) as [128, 2, mw]; broadcast over n rows
                    v = w["INV"][:, k * mw : k * mw + 2 * A * mw].rearrange(
                        "p (j c) -> p j c", j=2
                    )[:, :, :mw]
                    if n is None:
                        return v
                    return v[:, :, None, :].broadcast_to([128, 2, n, mw])

                # forward elimination
                for k in range(A):
                    # Pivot chain first, Pool's columns before DVE's own:
                    # |p|^2 + reciprocal run on Act + DVE for BOTH column
                    # blocks (Pool's ISA has no divide/reciprocal), and they
                    # must precede DVE's big update stream in DVE program
                    # order or Pool's step-k factors stall behind it.
                    # Interleave the per-column-block ops so consecutive DVE
                    # instructions never form a RAW pair (hides the ~95ns
                    # SBUF write-ack latency between dependent small ops).
                    for eng, c0, mw in reversed(engines()):
                        w = wt[c0]
                        HRe, HIm = sup[c0]
                        nc.scalar.square(w["TD"], row(HRe, k, k, mw))
                        nc.scalar.square(w["TU"], row(HIm, k, k, mw))
                    for eng, c0, mw in reversed(engines()):
                        w = wt[c0]
                        nc.vector.tensor_add(w["TD"], w["TD"], w["TU"])
                    for eng, c0, mw in reversed(engines()):
                        w = wt[c0]
                        nc.vector.reciprocal(w["TR"], w["TD"])
                    for eng, c0, mw in engines():
                        w = wt[c0]
                        HRe, HIm = sup[c0]
                        a = row(HRe, k, k, mw)
                        b_ = row(HIm, k, k, mw)
                        irk = w["INV"][:, k * mw : (k + 1) * mw]
                        iik = w["INV"][:, (A + k) * mw : (A + k + 1) * mw]
                        eng.tensor_mul(irk, a, w["TR"])
                        eng.tensor_mul(iik, b_, w["TR"])
                        n = A - 1 - k
                        if n == 0:
                            continue
                        # factors G = +H[i,k] * inv(p), via paired products:
                        #   PA = (a*ir || a*ii),  PB = (b*ir || b*ii)
                        car = rows3(HRe, k, k + 1, n, mw)
                        cai = rows3(HIm, k, k + 1, n, mw)
                        car4 = car[:, None, :, :].broadcast_to([128, 2, n, mw])
                        cai4 = cai[:, None, :, :].broadcast_to([128, 2, n, mw])

                        def sc4(T):
                            return T[:, : 2 * n * mw].rearrange(
                                "p (j r c) -> p j r c", j=2, r=n
                            )

                        def sc_half(T, h):
                            return T[:, h * n * mw : (h + 1) * n * mw]

                        eng.tensor_mul(sc4(w["PAs"]), car4, inv_pair(w, k, mw, n))
                        eng.tensor_mul(sc4(w["PBs"]), cai4, inv_pair(w, k, mw, n))
                        gre = w["GRe"][:, : n * mw]
                        gim = w["GIm"][:, : n * mw]
                        # gre = a*ir + b*ii, gim = b*ir - a*ii  (G = H[i,k]/p)
                        eng.tensor_add(gre, sc_half(w["PAs"], 0), sc_half(w["PBs"], 1))
                        eng.tensor_sub(gim, sc_half(w["PBs"], 0), sc_half(w["PAs"], 1))
                        # eliminate column k from planes k+1..7 and y, in
                        # groups of up to 4 planes per instruction: the plane
                        # index is a third AP dim (stride A*mw), so one
                        # [128, wg, n, mw] op covers wg planes.  H[i,j] -= G*B
                        # (complex), with products cycling through PAs/PBs.
                        js = list(range(k + 1, NP))
                        while js:
                            wg = min(4, len(js))
                            j0 = js[0]
                            js = js[wg:]

                            def wrows(T):
                                base = off(j0, k + 1, mw)
                                return T[:, base : base + wg * A * mw].rearrange(
                                    "p (w c) -> p w c", w=wg
                                )[:, :, : n * mw]

                            def wrow_b(T):
                                base = off(j0, k, mw)
                                v = T[:, base : base + wg * A * mw].rearrange(
                                    "p (w c) -> p w c", w=wg
                                )[:, :, :mw]
                                return v[:, :, None, :].broadcast_to(
                                    [128, wg, n, mw]
                                )

                            def fw(Ft):
                                v = Ft[:, : n * mw].rearrange(
                                    "p (r c) -> p r c", r=n
                                )
                                return v[:, None, :, :].broadcast_to(
                                    [128, wg, n, mw]
                                )

                            hr, hi = wrows(HRe), wrows(HIm)
                            Br, Bi = wrow_b(HRe), wrow_b(HIm)
                            grew, gimw = fw(w["GRe"]), fw(w["GIm"])
                            SA4 = w["PAs"][:, : wg * n * mw].rearrange(
                                "p (w r c) -> p w r c", w=wg, r=n
                            )
                            SA3 = w["PAs"][:, : wg * n * mw].rearrange(
                                "p (w c) -> p w c", w=wg
                            )
                            SB4 = w["PBs"][:, : wg * n * mw].rearrange(
                                "p (w r c) -> p w r c", w=wg, r=n
                            )
                            SB3 = w["PBs"][:, : wg * n * mw].rearrange(
                                "p (w c) -> p w c", w=wg
                            )
                            # H[i,j] -= G*B (complex); products regrouped by
                            # factor so consecutive ops never share a RAW
                            # destination (longer dep gaps -> less ack stall)
                            eng.tensor_mul(SA4, grew, Br)
                            eng.tensor_mul(SB4, grew, Bi)
                            eng.tensor_sub(hr, hr, SA3)
                            eng.tensor_sub(hi, hi, SB3)
                            eng.tensor_mul(SA4, gimw, Bi)
                            eng.tensor_mul(SB4, gimw, Br)
                            eng.tensor_add(hr, hr, SA3)
                            eng.tensor_sub(hi, hi, SB3)

                # back substitution (Jordan): x_k = y_k*invp, then clear col k
                for k in range(A - 1, -1, -1):
                    # xrow holds x_k full-width (re || im) so a single PE
                    # transpose per component can stage the store; bufs=2
                    # decouples consecutive k stores.
                    xrow = stgo.tile([128, 2 * M], F32, tag="xrow", name="xrow")
                    for eng, c0, mw in engines():
                        w = wt[c0]
                        HRe, HIm = sup[c0]
                        yr = row(HRe, 8, k, mw)
                        yi = row(HIm, 8, k, mw)
                        # p1 = (yr*ir || yr*ii), p2 = (yi*ir || yi*ii)
                        p1 = w["PAs"][:, : 2 * mw].rearrange("p (j c) -> p j c", j=2)
                        p2 = w["PBs"][:, : 2 * mw].rearrange("p (j c) -> p j c", j=2)
                        yr2 = yr[:, None, :].broadcast_to([128, 2, mw])
                        yi2 = yi[:, None, :].broadcast_to([128, 2, mw])
                        eng.tensor_mul(p1, yr2, inv_pair(w, k, mw))
                        eng.tensor_mul(p2, yi2, inv_pair(w, k, mw))
                        # x = y*conj(p)/|p|^2: xr = yr*ir + yi*ii,
                        #                      xi = yi*ir - yr*ii
                        eng.tensor_add(
                            xrow[:, c0 : c0 + mw],
                            w["PAs"][:, :mw], w["PBs"][:, mw : 2 * mw],
                        )
                        eng.tensor_sub(
                            xrow[:, M + c0 : M + c0 + mw],
                            w["PBs"][:, :mw], w["PAs"][:, mw : 2 * mw],
                        )
                    # x_k is final now -- store it while the rest of the back
                    # pass still runs on the elementwise engines.
                    so = stgo.tile([M, 2 * FS], F32, tag="so")
                    so3 = so.rearrange("p (f c) -> p f c", c=2)
                    for comp in range(2):
                        po = pso_pool.tile([M, FS], F32, tag="pso")
                        nc.tensor.transpose(
                            po, xrow[:, comp * M : (comp + 1) * M],
                            ident[:128, :128],
                        )
                        nc.scalar.copy(so3[:, :, comp], po)
                    dst = out[k, :, b0 : b0 + NB]
                    nc.sync.dma_start(dst, so)
                    if k == 0:
                        continue
                    for eng, c0, mw in engines():
                        w = wt[c0]
                        HRe, HIm = sup[c0]
                        xr_p = xrow[:, c0 : c0 + mw]
                        xi_p = xrow[:, M + c0 : M + c0 + mw]
                        cr = rows3(HRe, k, 0, k, mw)
                        ci_ = rows3(HIm, k, 0, k, mw)
                        xrB = xr_p[:, None, :].broadcast_to([128, k, mw])
                        xiB = xi_p[:, None, :].broadcast_to([128, k, mw])

                        def sc3(T):
                            return T[:, : k * mw].rearrange("p (r c) -> p r c", r=k)

                        qa, qb, qc, qd = (
                            sc3(w[t]) for t in ("PAs", "PBs", "PCs", "PDs")
                        )
                        eng.tensor_mul(qa, cr, xrB)
                        eng.tensor_mul(qb, ci_, xiB)
                        eng.tensor_mul(qc, cr, xiB)
                        eng.tensor_mul(qd, ci_, xrB)
                        ytr = rows3(HRe, 8, 0, k, mw)
                        yti = rows3(HIm, 8, 0, k, mw)
                        # y_i -= H[i,k] * x_k
                        eng.tensor_sub(ytr, ytr, qa)
                        eng.tensor_add(ytr, ytr, qb)
                        eng.tensor_sub(yti, yti, qc)
                        eng.tensor_sub(yti, yti, qd)

    nc.finalize()
    return nc


_NC_CACHE = None


def _get_nc():
    global _NC_CACHE
    if _NC_CACHE is None:
        _NC_CACHE = _build()
    return _NC_CACHE


def _prep_core(y_re, y_im, h_re, h_im, c):
    """Host-side shard prep for core c: f-slice + block-diagonal extraction."""
    fsl = slice(c * FS, (c + 1) * FS)
    ue = np.arange(U)
    maps = {}
    for name, h in (("hd_re", h_re), ("hd_im", h_im)):
        h6 = h[:, 0, :, :, :, :, fsl].reshape(B, U, A, U, A, S, FS)
        hd = h6[:, ue, :, ue]              # [u, b, i, j, s, f]
        maps[name] = np.ascontiguousarray(
            hd.transpose(3, 0, 1, 4, 2, 5), dtype=np.float32
        )                                   # [j, u, b, s, i, f]
    for name, y in (("yd_re", y_re), ("yd_im", y_im)):
        y5 = y[:, 0, :, :, fsl].reshape(B, U, A, S, FS)   # [b, u, i, s, f]
        maps[name] = np.ascontiguousarray(
            y5.transpose(2, 1, 0, 3, 4), dtype=np.float32
        )                                   # [i, u, b, s, f]
    return maps


def kernel(y_re, y_im, h_re, h_im, **_ignored):
    global LAST_RESULTS
    y_re = np.asarray(y_re, dtype=np.float32)
    y_im = np.asarray(y_im, dtype=np.float32)
    h_re = np.asarray(h_re, dtype=np.float32)
    h_im = np.asarray(h_im, dtype=np.float32)

    nc = _get_nc()
    in_maps = [_prep_core(y_re, y_im, h_re, h_im, c) for c in range(NCORES)]
    trace = bool(int(os.environ.get("BD_TRACE", "0")))
    res = run_bass_kernel_spmd(
        nc, in_maps, core_ids=list(range(NCORES)), trace=trace
    )
    LAST_RESULTS = res
    outs = []
    for r in res.results:
        o = r["out"]                              # [i, u, b, s, f, c]
        o = o.transpose(2, 1, 0, 3, 4, 5)         # [b, u, i, s, f, c]
        outs.append(o.reshape(B, NR, S, FS, 2))
    full = np.concatenate(outs, axis=3)           # [B, NR, S, F, 2]
    return np.ascontiguousarray(full[:, None])    # [B, 1, NR, S, F, 2]
